# revision 13
# baseline (speedup 1.0000x reference)
"""Cost-volume kernel for TRN2 (one NeuronCore processes one batch element).

out[(40 - 9a - b) % 81, h, w] = (1/81) * sum_c x1[c, h, w] * x2p[c, h+a, w+b]
for a, b in [0, 9), where x2p is x2 zero-padded by 4 on each h/w edge.

Structure per (h row, 128-wide w chunk):
  - One 3-bank PSUM tile [128, 1536]: 3 a-triplet matmuls (M=128 pixels,
    N=408 = 3 rows x 136 window, fp32r, K=192 in 2 c-tiles) at 512-aligned
    offsets.
  - 4 compacting ACT/DVE copies (one per 32-pixel group, scale 1/81):
    gsb[32g+m, (a, j)] = psum[32g+m, a-block, 32g + j], j in [0, 40).
  - Dump gsb [128, 360] -> DRAM scratch (dense).
  - Shear re-read from DRAM (flat affine strides) -> SH[p, 9a+b] =
    scr[p, 40a + (p%32) + b]; then 2 reversed DVE copies reorder
    j = 9a+b -> ch = (40-j) % 81.
  - PE transpose SH2 -> psum_tr [81, 128]; DVE copy into row staging.
  - 1 DMA per row stores staging -> out[:, h, :].
"""

import numpy as np
from contextlib import ExitStack

import concourse.bacc as bacc
import concourse.bass as bass
import concourse.tile as tile
from concourse import mybir

F32 = mybir.dt.float32
F32R = mybir.dt.float32r
BF16 = mybir.dt.bfloat16

C, H, W = 192, 128, 256
R, D, NCH = 4, 9, 81
BAND = 32          # h rows per band
GW = 32            # pixels per matmul group (col-tiling granularity)
WIN = GW + 8       # w' window per group
NFREE = D * WIN    # psum free size = 360
XROWS = BAND + 8   # x2 rows resident per band
XW = W + 8         # padded x2 row length

# ch = (40 - 9a - b) % 81 split into affine rectangles (a, b0, nb, ch_at_b0)
SHEAR_RECTS = [(a, 0, 9, (40 - 9 * a) % NCH) for a in range(9)]
SHEAR_RECTS[4:5] = [(4, 0, 5, 4), (4, 5, 4, 80)]


def build_cv_kernel(H_=H, use_f32r=True, reps=1):
    nbands = H_ // BAND
    nc = bacc.Bacc("TRN2", target_bir_lowering=False, debug=False)
    xdt = BF16
    gdt = BF16  # gsb / scratch / shear dtype
    x1 = nc.dram_tensor("x1", [C, H_, W], xdt, kind="ExternalInput").ap()
    x2 = nc.dram_tensor("x2", [C, H_, W], xdt, kind="ExternalInput").ap()
    ident = nc.dram_tensor("ident", [128, 128], BF16, kind="ExternalInput").ap()
    out = nc.dram_tensor("out", [NCH, H_, W], F32, kind="ExternalOutput").ap()
    # scratch[h, wc, p, f]
    scr = nc.dram_tensor("scr", [H_, 2, 128, NFREE], gdt).ap()

    with tile.TileContext(nc) as tc, ExitStack() as ctx:
        const_pool = ctx.enter_context(tc.tile_pool(name="const", bufs=1))
        x_pool = ctx.enter_context(tc.tile_pool(name="x", bufs=2))
        g_pool = ctx.enter_context(tc.tile_pool(name="g", bufs=3))
        o_pool = ctx.enter_context(tc.tile_pool(name="o", bufs=3))
        ps_gram = ctx.enter_context(tc.tile_pool(name="psg", bufs=2, space="PSUM"))
        ps_tr = ctx.enter_context(tc.tile_pool(name="pst", bufs=2, space="PSUM"))

        idt = const_pool.tile([128, 128], BF16)
        nc.sync.dma_start(idt[:], ident[:])

        for rep in range(reps):
          for band in range(nbands):
            h0 = band * BAND
            # ---- x1 band: [c, BAND, W] in two c tiles
            x1lo = x_pool.tile([128, BAND * W], xdt, tag="x1lo")
            x1hi = x_pool.tile([64, BAND * W], xdt, tag="x1hi")
            nc.sync.dma_start(
                x1lo[:].rearrange("p (r w) -> p r w", r=BAND),
                x1[0:128, h0 : h0 + BAND, :],
            )
            nc.sync.dma_start(
                x1hi[:].rearrange("p (r w) -> p r w", r=BAND),
                x1[128:192, h0 : h0 + BAND, :],
            )

            # ---- x2 band: padded rows [c, XROWS, XW]; local row t = x2p row
            #      h0 + t = orig x2 row h0 + t - 4.
            x2lo = x_pool.tile([128, XROWS * XW], xdt, tag="x2lo")
            x2hi = x_pool.tile([64, XROWS * XW], xdt, tag="x2hi")
            r_lo3 = x2lo[:].rearrange("p (r w) -> p r w", r=XROWS)
            r_hi3 = x2hi[:].rearrange("p (r w) -> p r w", r=XROWS)
            for r3 in (r_lo3, r_hi3):
                nc.gpsimd.memset(r3[:, :, 0:4].bitcast(F32), 0.0)
                nc.gpsimd.memset(r3[:, :, W + 4 : XW].bitcast(F32), 0.0)
            t_first = max(0, 4 - h0)
            t_last = min(XROWS, H_ + 4 - h0)
            if t_first > 0:
                nc.gpsimd.memset(r_lo3[:, 0:t_first, :].bitcast(F32), 0.0)
                nc.gpsimd.memset(r_hi3[:, 0:t_first, :].bitcast(F32), 0.0)
            if t_last < XROWS:
                nc.gpsimd.memset(r_lo3[:, t_last:XROWS, :].bitcast(F32), 0.0)
                nc.gpsimd.memset(r_hi3[:, t_last:XROWS, :].bitcast(F32), 0.0)
            o_first = h0 - 4 + t_first
            o_last = h0 - 4 + t_last
            nc.sync.dma_start(
                r_lo3[:, t_first:t_last, 4 : W + 4], x2[0:128, o_first:o_last, :]
            )
            nc.sync.dma_start(
                r_hi3[:, t_first:t_last, 4 : W + 4], x2[128:192, o_first:o_last, :]
            )

            for h_loc in range(BAND):
                h = h0 + h_loc
                outst = o_pool.tile([NCH, W], F32, tag="outst")
                for wc in range(2):
                    # 3 a-triplet matmul outputs [128, 408] at 512-aligned
                    # offsets of one 3-bank psum tile; M=128, fp32r, N=408.
                    gram = ps_gram.tile([128, 1536], F32, tag="gram")
                    for t in range(3):
                        for k, (x1t, x2t) in enumerate(
                            ((x1lo, x2lo), (x1hi, x2hi))
                        ):
                            ncc = x1t[:].ap[0][1]  # 128 or 64
                            lhsT = bass.AP(
                                tensor=x1t[:].tensor,
                                offset=h_loc * W + wc * 128,
                                ap=[[BAND * W, ncc], [1, 128]],
                            )
                            rhs = bass.AP(
                                tensor=x2t[:].tensor,
                                offset=(h_loc + 3 * t) * XW + wc * 128,
                                ap=[[XROWS * XW, ncc], [XW, 3], [1, 136]],
                            )
                            nc.tensor.matmul(
                                gram[:, 512 * t : 512 * t + 408],
                                lhsT,
                                rhs,
                                start=(k == 0),
                                stop=(k == 1),
                            )
                    # compacting psum -> SBUF copies with 1/81 scale: group g
                    # keeps window [32g, 32g+40) of each a-block.
                    gsb = g_pool.tile([128, NFREE], gdt, tag="gsb")
                    gpp = gram[:].ap[0][0]
                    gbp = gsb[:].ap[0][0]
                    for g in range(4):
                        src = bass.AP(
                            tensor=gram[:].tensor,
                            offset=GW * g * gpp + GW * g,
                            ap=[[gpp, GW], [512, 3], [136, 3], [1, WIN]],
                        )
                        dst = bass.AP(
                            tensor=gsb[:].tensor,
                            offset=GW * g * gbp,
                            ap=[[gbp, GW], [3 * WIN, 3], [WIN, 3], [1, WIN]],
                        )
                        if g < 2:
                            nc.scalar.mul(dst, src, 1.0 / NCH)
                        else:
                            nc.vector.tensor_scalar_mul(dst, src, 1.0 / NCH)
                    # dense dump to DRAM scratch
                    nc.sync.dma_start(scr[h, wc], gsb[:])
                    # shear re-read: SH[32g + m, ch(a,b)] = scr[h, wc, 32g+m,
                    #   40a + m + b]; src flat elem idx = 11520g + 361m + 40a + b
                    sh = g_pool.tile([128, NCH], gdt, tag="sh")
                    sp = sh[:].ap[0][0]
                    scr_base = (h * 2 + wc) * 128 * NFREE
                    for a in range(D):
                        src = bass.AP(
                            tensor=scr.tensor,
                            offset=scr_base + WIN * a,
                            ap=[[GW * NFREE, 4], [NFREE + 1, GW], [1, D]],
                        )
                        dst = bass.AP(
                            tensor=sh[:].tensor,
                            offset=D * a,
                            ap=[[sp, 128], [1, D]],
                        )
                        nc.sync.dma_start(dst, src)
                    # reorder j = 9a+b -> ch = (40 - j) % 81 (2 reversed copies)
                    sh2 = g_pool.tile([128, NCH], gdt, tag="sh2")
                    s2p = sh2[:].ap[0][0]
                    for j0, n, c0 in ((0, 41, 40), (41, 40, 80)):
                        nc.vector.tensor_copy(
                            bass.AP(tensor=sh2[:].tensor, offset=c0,
                                    ap=[[s2p, 128], [-1, n]]),
                            bass.AP(tensor=sh[:].tensor, offset=j0,
                                    ap=[[sp, 128], [1, n]]),
                        )
                    # transpose [128, 81] -> [81, 128] on PE
                    ptr = ps_tr.tile([NCH, 128], BF16, tag="ptr")
                    nc.tensor.transpose(ptr[:], sh2[:], idt[:])
                    nc.scalar.mul(
                        outst[:, wc * 128 : wc * 128 + 128], ptr[:], 1.0
                    )
                # store row h (staging partitions already in ch order)
                nc.sync.dma_start(
                    bass.AP(
                        tensor=out.tensor,
                        offset=h * W,
                        ap=[[H_ * W, NCH], [1, W]],
                    ),
                    outst[:],
                )
    nc.compile()
    return nc


def ref_one(x1, x2):
    """numpy reference for one batch element: x1, x2 [C, H, W]."""
    C_, H_, W_ = x1.shape
    x2p = np.pad(x2, ((0, 0), (4, 4), (4, 4)))
    out = np.zeros((NCH, H_, W_), np.float32)
    for a in range(9):
        for b in range(9):
            ch = (40 - 9 * a - b) % NCH
            out[ch] = (x1 * x2p[:, a : a + H_, b : b + W_]).sum(0) / NCH
    return out




_NC_CACHE = {}


def _get_nc():
    if "nc" not in _NC_CACHE:
        _NC_CACHE["nc"] = build_cv_kernel(H_=H)
    return _NC_CACHE["nc"]


def make_in_maps(x1, x2):
    import ml_dtypes

    bf16 = ml_dtypes.bfloat16
    x1 = np.ascontiguousarray(np.asarray(x1, dtype=np.float32).astype(bf16))
    x2 = np.ascontiguousarray(np.asarray(x2, dtype=np.float32).astype(bf16))
    eye = np.eye(128, dtype=np.float32).astype(bf16)
    return [{"x1": x1[i], "x2": x2[i], "ident": eye} for i in range(x1.shape[0])]


def kernel(x1, x2):
    """Full-input entry point: x1, x2 [8, 192, 128, 256] float32 ->
    [8, 81, 128, 256] float32. Data-parallel over batch: core i computes
    batch element i."""
    from concourse import bass_utils

    B = np.asarray(x1).shape[0]
    assert np.asarray(x1).shape == (B, C, H, W)
    nc = _get_nc()
    in_maps = make_in_maps(x1, x2)
    res = bass_utils.run_bass_kernel_spmd(nc, in_maps, core_ids=list(range(B)))
    return np.stack([res.results[i]["out"] for i in range(B)], axis=0)

